# revision 31
# baseline (speedup 1.0000x reference)
"""Multi-head attention (B=2, S=2048, H=1024, 16 heads) on 8 trn2 NeuronCores.

Sharding: 2-way batch x 4-way head-group tensor parallel. Core c handles
batch c//4 and heads 4*(c%4)..4*(c%4)+3 (256 channels of the QKV
projections / 256 input channels of the output projection). Each core
consumes only its batch's activations (halves HBM traffic vs full
replication); the 4 partial wo outputs per batch are summed on the host.

Device-side dataflow per core (bf16 matmuls, f32 PSUM):
  QT/KT[c, s]   : transposed projections, channels on partitions
  VT[c, s] -> V : PE-transposed to natural layout, ones-augmented (65 cols)
  S^T[k, q]     = KT_h^T-tile . QT_h              (per head, 128-key tiles)
  E = exp(S/8)  (no max subtraction: scores ~ N(0,1))
  O^T[65, q]    accumulates V_aug^T . E over 16 key tiles (row 64 = sums)
  r = 1/sums    on one partition row; broadcast via a K=1 matmul
  On = O * r    ; y^T partial = wo_c^T . On, DMA'd straight from PSUM

The emission interleaves the second half of the projections, the
normalization matmuls and the output projection into the attention
stream so the PE never idles (idle gaps drop it out of max p-state).
"""

import os
import threading

import numpy as np
import ml_dtypes

import concourse.bass as bass
import concourse.mybir as mybir
import concourse.tile as tile
from concourse import bacc
from concourse.bass_utils import run_bass_kernel_spmd

BF16 = ml_dtypes.bfloat16
F32 = mybir.dt.float32
BF = mybir.dt.bfloat16

B = 2
S = 2048
H = 1024
NH_LOCAL = 4        # heads per core
HD = 64             # head dim
CPC = 256           # channels per core
NF = H // 128       # contraction chunks
NKT = S // 128      # key tiles
NQB = 2             # q blocks of 1024
QB = S // NQB
N_CORES = 8

_cache = threading.Lock()
_nc = None

LAST_RESULT = None  # BassKernelResults of the most recent run (for test.py)


def _build_nc():
    nc = bacc.Bacc(None, target_bir_lowering=False, debug=False)

    xq_d = nc.dram_tensor("xq_t", [H, S], BF, kind="ExternalInput")
    xk_d = nc.dram_tensor("xk_t", [H, S], BF, kind="ExternalInput")
    xv_d = nc.dram_tensor("xv_t", [H, S], BF, kind="ExternalInput")
    wq_d = nc.dram_tensor("wq_t", [H, CPC], BF, kind="ExternalInput")
    wk_d = nc.dram_tensor("wk_t", [H, CPC], BF, kind="ExternalInput")
    wv_d = nc.dram_tensor("wv_t", [H, CPC], BF, kind="ExternalInput")
    bq_d = nc.dram_tensor("bq", [128, 2], F32, kind="ExternalInput")
    bk_d = nc.dram_tensor("bk", [128, 2], F32, kind="ExternalInput")
    bv_d = nc.dram_tensor("bv", [128, 2], F32, kind="ExternalInput")
    wo_d = nc.dram_tensor("wo_t", [CPC, H], BF, kind="ExternalInput")
    id_d = nc.dram_tensor("ident", [128, 128], BF, kind="ExternalInput")
    y_d = nc.dram_tensor("y_t", [H, S], BF, kind="ExternalOutput")

    xq_ap = xq_d.rearrange("(f p) s -> f p s", p=128)
    xk_ap = xk_d.rearrange("(f p) s -> f p s", p=128)
    xv_ap = xv_d.rearrange("(f p) s -> f p s", p=128)
    y_ap = y_d.rearrange("(oc p) s -> oc p s", p=128)

    Exp = mybir.ActivationFunctionType.Exp
    Copy = mybir.ActivationFunctionType.Identity

    with tile.TileContext(nc) as tc:
        with (
            tc.tile_pool(name="const", bufs=1) as const,
            tc.tile_pool(name="res", bufs=1) as res,
            tc.tile_pool(name="work", bufs=4) as work,
        ):
            # --- constants / weights ---
            wq_sb = const.tile([128, NF, CPC], BF)
            wk_sb = const.tile([128, NF, CPC], BF)
            wv_sb = const.tile([128, NF, CPC], BF)
            wo_sb = const.tile([128, 2, NF, 128], BF)
            bq_sb = const.tile([128, 2], F32)
            bk_sb = const.tile([128, 2], F32)
            bv_sb = const.tile([128, 2], F32)
            id_sb = const.tile([128, 128], BF)
            nc.sync.dma_start(wq_sb[:], wq_d.rearrange("(f p) c -> p f c", p=128))
            nc.sync.dma_start(wk_sb[:], wk_d.rearrange("(f p) c -> p f c", p=128))
            nc.sync.dma_start(wv_sb[:], wv_d.rearrange("(f p) c -> p f c", p=128))
            nc.sync.dma_start(wo_sb[:], wo_d.rearrange("(hf p) (oc c) -> p hf oc c", p=128, c=128))
            nc.sync.dma_start(bq_sb[:], bq_d[:])
            nc.sync.dma_start(bk_sb[:], bk_d[:])
            nc.sync.dma_start(bv_sb[:], bv_d[:])
            nc.sync.dma_start(id_sb[:], id_d[:])

            # --- residents ---
            QT = res.tile([128, 2, S], BF)     # [p, chan-half, tok]
            KT = res.tile([128, 2, S], BF)
            VT = res.tile([128, 2, S], BF)
            V = res.tile([128, NKT, NH_LOCAL, HD + 1], BF)  # natural + ones
            On = res.tile([128, 2, NQB, QB], BF)            # normalized attn out
            nc.gpsimd.memset(V[:, :, :, HD : HD + 1], 1.0)

            # input activations, 8 chunks of [128, S] each, in consumption order
            xq_sb = res.tile([128, NF, S], BF)
            xk_sb = res.tile([128, NF, S], BF)
            xv_sb = res.tile([128, NF, S], BF)
            for x_sb, x_ap in ((xq_sb, xq_ap), (xk_sb, xk_ap), (xv_sb, xv_ap)):
                for f in range(NF):
                    nc.sync.dma_start(x_sb[:, f, :], x_ap[f])

            # matmul psum outputs are limited to one bank (512 f32 cols)
            def mm512(out, lhsT, rhs, **kw):
                n = rhs.shape[-1]
                for j in range(0, n, 512):
                    w = min(512, n - j)
                    nc.tensor.matmul(
                        out[:, j : j + w], lhsT=lhsT, rhs=rhs[:, j : j + w],
                        **kw,
                    )

            with tc.tile_pool(name="psA", bufs=4, space="PSUM") as psA:
                # --- Q/K/V^T projections: consume each x chunk once across
                # all four (chan-half, q-half) quadrants so the PE tracks the
                # chunk DMAs instead of waiting for the full tensor ---
                def proj(name, x_sb, w_sb, b_sb, out_t):
                    pps = {}
                    for hf in range(2):
                        for qh in range(2):
                            pps[(hf, qh)] = psA.tile(
                                [128, QB], F32, tag="pp",
                                name=f"pp_{name}{hf}{qh}",
                            )
                    for f in range(NF):
                        for hf in range(2):
                            for qh in range(2):
                                cs = slice(qh * QB, (qh + 1) * QB)
                                mm512(
                                    pps[(hf, qh)],
                                    lhsT=w_sb[:, f, hf * 128 : (hf + 1) * 128],
                                    rhs=x_sb[:, f, cs],
                                    start=(f == 0),
                                    stop=(f == NF - 1),
                                )
                    for hf in range(2):
                        for qh in range(2):
                            cs = slice(qh * QB, (qh + 1) * QB)
                            # on DVE, not Act: the scalar engine must stay
                            # free to run ahead on the attention exps
                            nc.vector.tensor_scalar_add(
                                out_t[:, hf, cs], pps[(hf, qh)][:],
                                b_sb[:, hf : hf + 1],
                            )

                proj("q", xq_sb, wq_sb, bq_sb, QT)
                proj("k", xk_sb, wk_sb, bk_sb, KT)
                proj("v", xv_sb, wv_sb, bv_sb, VT)

            # --- attention + normalize + output projection, interleaved ---
            with tc.tile_pool(name="psB", bufs=2, space="PSUM") as psB:
                units = [(h, qb) for qb in range(NQB) for h in range(NH_LOCAL)]

                def emit_scores(h, qb, kt, e_list):
                    rows = slice(64 * (h % 2), 64 * (h % 2) + 64)
                    s_t = psB.tile([128, QB], F32, tag="s",
                                   name=f"s_{h}{qb}{kt}")
                    mm512(
                        s_t,
                        lhsT=KT[rows, h // 2, kt * 128 : (kt + 1) * 128],
                        rhs=QT[rows, h // 2, qb * QB : (qb + 1) * QB],
                    )
                    e_t = work.tile([128, QB], BF, tag="e",
                                    name=f"e_{h}{qb}{kt}", bufs=10)
                    nc.scalar.activation(e_t[:], s_t[:], Exp, scale=0.125)
                    e_list.append(e_t)

                # entry stream: V transposes interleaved with prebuilt
                # scores+exp for unit (0,0) so Act builds a lead
                e00 = []
                for i in range(8):
                    for j in range(2):
                        hf, tt = divmod(2 * i + j, NKT)
                        tp = psB.tile([128, 128], BF, tag="s",
                                      name=f"tp_{hf}{tt}")
                        nc.tensor.transpose(
                            tp[:], VT[:, hf, tt * 128 : (tt + 1) * 128], id_sb[:]
                        )
                        nc.vector.tensor_copy(V[:, tt, 2 * hf, 0:HD], tp[:, 0:HD])
                        nc.vector.tensor_copy(
                            V[:, tt, 2 * hf + 1, 0:HD], tp[:, HD:128]
                        )
                    emit_scores(0, 0, i, e00)
                for i in range(8):
                    for j in range(2):
                        hf, tt = divmod(16 + 2 * i + j, NKT)
                        tp = psB.tile([128, 128], BF, tag="s",
                                      name=f"tp_{hf}{tt}")
                        nc.tensor.transpose(
                            tp[:], VT[:, hf, tt * 128 : (tt + 1) * 128], id_sb[:]
                        )
                        nc.vector.tensor_copy(V[:, tt, 2 * hf, 0:HD], tp[:, 0:HD])
                        nc.vector.tensor_copy(
                            V[:, tt, 2 * hf + 1, 0:HD], tp[:, HD:128]
                        )

                def normalize(h, qb):
                    """emit copy->shift->recip->broadcast->mul for unit"""
                    o_t, odd = o_tiles[(h, qb)]
                    rr = work.tile([HD + 1, QB], F32, tag="rr",
                                   name=f"rr_{h}{qb}", bufs=1)
                    nc.vector.tensor_copy(
                        rr[HD : HD + 1, :], o_t[HD : HD + 1, :]
                    )
                    rr0 = work.tile([1, QB], F32, tag="rr0",
                                    name=f"rr0_{h}{qb}", bufs=1)
                    nc.scalar.dma_start(rr0[:], rr[HD : HD + 1, :])
                    rrc = work.tile([1, QB], F32, tag="rrc",
                                    name=f"rrc_{h}{qb}", bufs=1)
                    nc.vector.reciprocal_approx_fast(rrc[:], rr0[:])
                    rbb = work.tile([HD, QB], F32, tag="rbb",
                                    name=f"rbb_{h}{qb}", bufs=1)
                    nc.gpsimd.partition_broadcast(rbb[:], rrc[:])
                    if not odd:
                        nc.vector.tensor_mul(
                            On[0:HD, h // 2, qb, :], o_t[0:HD, :], rbb[:]
                        )
                    else:
                        ot = work.tile([HD, QB], BF, tag="ot",
                                       name=f"ot_{h}{qb}", bufs=1)
                        nc.vector.tensor_mul(ot[:], o_t[0:HD, :], rbb[:])
                        nc.scalar.dma_start(On[HD:128, h // 2, qb, :], ot[:])

                def outproj_step(qb, oc):
                    py = psB.tile([128, QB], F32, tag="s", name=f"py_{qb}{oc}")
                    for hf in range(2):
                        mm512(
                            py,
                            lhsT=wo_sb[:, hf, oc, :],
                            rhs=On[:, hf, qb, :],
                            start=(hf == 0),
                            stop=(hf == 1),
                        )
                    ysb = work.tile([128, QB], BF, tag="y", name=f"y_{qb}{oc}",
                                    bufs=4)
                    nc.vector.tensor_copy(ysb[:], py[:])
                    eng = (nc.sync, nc.scalar, nc.gpsimd)[oc % 3]
                    eng.dma_start(
                        y_ap[oc, :, qb * QB : (qb + 1) * QB], ysb[:]
                    )

                o_tiles = {}
                pending_norm = []
                pending_out = []
                for h, qb in units:
                    o_t = psB.tile([HD + 1, QB], F32, tag="o", name=f"o_{h}{qb}")
                    o_tiles[(h, qb)] = (o_t, h % 2 == 1)
                    e_tiles = e00 if (h, qb) == (0, 0) else []
                    for kt in range(NKT + 1):
                        if kt < NKT and len(e_tiles) <= kt:
                            emit_scores(h, qb, kt, e_tiles)
                        # interleave deferred work into the PE stream
                        if kt == 2 and pending_norm:
                            normalize(*pending_norm.pop(0))
                        if kt in (3, 5, 7, 9, 11, 13, 14, 15) and pending_out:
                            outproj_step(*pending_out.pop(0))
                        if kt >= 1:
                            k0 = kt - 1
                            mm512(
                                o_t,
                                lhsT=V[:, k0, h, :],
                                rhs=e_tiles[k0][:],
                                start=(k0 == 0),
                                stop=(k0 == NKT - 1),
                            )
                    pending_norm.append((h, qb))
                    if h == NH_LOCAL - 1:
                        pending_out.extend((qb, oc) for oc in range(NF))
                # drain
                while pending_norm:
                    normalize(*pending_norm.pop(0))
                while pending_out:
                    outproj_step(*pending_out.pop(0))
    nc.compile()
    return nc


def _get_nc():
    global _nc
    with _cache:
        if _nc is None:
            _nc = _build_nc()
        return _nc


def kernel(q, k, v, wq_w, wq_b, wk_w, wk_b, wv_w, wv_b, wo_w, wo_b):
    global LAST_RESULT
    nc = _get_nc()

    q = np.asarray(q, dtype=np.float32)
    k = np.asarray(k, dtype=np.float32)
    v = np.asarray(v, dtype=np.float32)
    wq_w = np.asarray(wq_w, dtype=np.float32)
    wk_w = np.asarray(wk_w, dtype=np.float32)
    wv_w = np.asarray(wv_w, dtype=np.float32)
    wo_w = np.asarray(wo_w, dtype=np.float32)

    def xT(a, b):
        return np.ascontiguousarray(a[b].astype(BF16).T)

    def b2(a, cs):
        return np.ascontiguousarray(
            np.asarray(a, np.float32)[cs].reshape(2, 128).T
        )

    ident = np.eye(128, dtype=BF16)

    in_maps = []
    for c in range(N_CORES):
        b = c // 4
        hg = c % 4
        cs = slice(hg * CPC, (hg + 1) * CPC)
        in_maps.append({
            "xq_t": xT(q, b),
            "xk_t": xT(k, b),
            "xv_t": xT(v, b),
            "wq_t": np.ascontiguousarray(wq_w[cs, :].astype(BF16).T),
            "wk_t": np.ascontiguousarray(wk_w[cs, :].astype(BF16).T),
            "wv_t": np.ascontiguousarray(wv_w[cs, :].astype(BF16).T),
            "bq": b2(wq_b, cs),
            "bk": b2(wk_b, cs),
            "bv": b2(wv_b, cs),
            "wo_t": np.ascontiguousarray(wo_w[:, cs].astype(BF16).T),
            "ident": ident,
        })

    res = run_bass_kernel_spmd(
        nc, in_maps, core_ids=list(range(N_CORES)),
        trace=bool(int(os.environ.get("MHA_TRACE", "0"))),
    )
    LAST_RESULT = res

    ys = []
    for b in range(B):
        y = res.results[b * 4]["y_t"].astype(np.float64)
        for hg in range(1, 4):
            y += res.results[b * 4 + hg]["y_t"]
        ys.append(y.T)
    y = np.stack(ys) + np.asarray(wo_b, np.float64)[None, None, :]
    return y.astype(np.float32)


# revision 33
# speedup vs baseline: 1.0309x; 1.0309x over previous
"""Multi-head attention (B=2, S=2048, H=1024, 16 heads) on 8 trn2 NeuronCores.

Sharding: 2-way batch x 4-way head-group tensor parallel. Core c handles
batch c//4 and heads 4*(c%4)..4*(c%4)+3 (256 channels of the QKV
projections / 256 input channels of the output projection). Each core
consumes only its batch's activations (halves HBM traffic vs full
replication); the 4 partial wo outputs per batch are summed on the host.

Device-side dataflow per core (bf16 matmuls, f32 PSUM):
  QT/KT[c, s]   : transposed projections, channels on partitions
  VT[c, s] -> V : PE-transposed to natural layout, ones-augmented (65 cols)
  S^T[k, q]     = KT_h^T-tile . QT_h              (per head, 128-key tiles)
  E = exp(S/8)  (no max subtraction: scores ~ N(0,1))
  O^T[65, q]    accumulates V_aug^T . E over 16 key tiles (row 64 = sums)
  r = 1/sums    on one partition row; broadcast via a K=1 matmul
  On = O * r    ; y^T partial = wo_c^T . On, DMA'd straight from PSUM

The emission interleaves the second half of the projections, the
normalization matmuls and the output projection into the attention
stream so the PE never idles (idle gaps drop it out of max p-state).
"""

import os
import threading

import numpy as np
import ml_dtypes

import concourse.bass as bass
import concourse.mybir as mybir
import concourse.tile as tile
from concourse import bacc
from concourse.bass_utils import run_bass_kernel_spmd

BF16 = ml_dtypes.bfloat16
F32 = mybir.dt.float32
BF = mybir.dt.bfloat16

B = 2
S = 2048
H = 1024
NH_LOCAL = 4        # heads per core
HD = 64             # head dim
CPC = 256           # channels per core
NF = H // 128       # contraction chunks
NKT = S // 128      # key tiles
NQB = 2             # q blocks of 1024
QB = S // NQB
N_CORES = 8

_cache = threading.Lock()
_nc = None

LAST_RESULT = None  # BassKernelResults of the most recent run (for test.py)


def _build_nc():
    nc = bacc.Bacc(None, target_bir_lowering=False, debug=False)

    xq_d = nc.dram_tensor("xq_t", [H, S], BF, kind="ExternalInput")
    xk_d = nc.dram_tensor("xk_t", [H, S], BF, kind="ExternalInput")
    xv_d = nc.dram_tensor("xv_t", [H, S], BF, kind="ExternalInput")
    wq_d = nc.dram_tensor("wq_t", [H, CPC], BF, kind="ExternalInput")
    wk_d = nc.dram_tensor("wk_t", [H, CPC], BF, kind="ExternalInput")
    wv_d = nc.dram_tensor("wv_t", [H, CPC], BF, kind="ExternalInput")
    bq_d = nc.dram_tensor("bq", [128, 2], F32, kind="ExternalInput")
    bk_d = nc.dram_tensor("bk", [128, 2], F32, kind="ExternalInput")
    bv_d = nc.dram_tensor("bv", [128, 2], F32, kind="ExternalInput")
    wo_d = nc.dram_tensor("wo_t", [CPC, H], BF, kind="ExternalInput")
    id_d = nc.dram_tensor("ident", [128, 128], BF, kind="ExternalInput")
    y_d = nc.dram_tensor("y_t", [H, S], BF, kind="ExternalOutput")

    xq_ap = xq_d.rearrange("(f p) s -> f p s", p=128)
    xk_ap = xk_d.rearrange("(f p) s -> f p s", p=128)
    xv_ap = xv_d.rearrange("(f p) s -> f p s", p=128)
    y_ap = y_d.rearrange("(oc p) s -> oc p s", p=128)

    Exp = mybir.ActivationFunctionType.Exp
    Copy = mybir.ActivationFunctionType.Identity

    with tile.TileContext(nc) as tc:
        with (
            tc.tile_pool(name="const", bufs=1) as const,
            tc.tile_pool(name="res", bufs=1) as res,
            tc.tile_pool(name="work", bufs=4) as work,
        ):
            # --- constants / weights ---
            wq_sb = const.tile([128, NF, CPC], BF)
            wk_sb = const.tile([128, NF, CPC], BF)
            wv_sb = const.tile([128, NF, CPC], BF)
            wo_sb = const.tile([128, 2, NF, 128], BF)
            bq_sb = const.tile([128, 2], F32)
            bk_sb = const.tile([128, 2], F32)
            bv_sb = const.tile([128, 2], F32)
            id_sb = const.tile([128, 128], BF)
            nc.sync.dma_start(wq_sb[:], wq_d.rearrange("(f p) c -> p f c", p=128))
            nc.sync.dma_start(wk_sb[:], wk_d.rearrange("(f p) c -> p f c", p=128))
            nc.sync.dma_start(wv_sb[:], wv_d.rearrange("(f p) c -> p f c", p=128))
            nc.sync.dma_start(wo_sb[:], wo_d.rearrange("(hf p) (oc c) -> p hf oc c", p=128, c=128))
            nc.sync.dma_start(bq_sb[:], bq_d[:])
            nc.sync.dma_start(bk_sb[:], bk_d[:])
            nc.sync.dma_start(bv_sb[:], bv_d[:])
            nc.sync.dma_start(id_sb[:], id_d[:])

            # --- residents ---
            QT = res.tile([128, 2, S], BF)     # [p, chan-half, tok]
            KT = res.tile([128, 2, S], BF)
            VT = res.tile([128, 2, S], BF)
            V = res.tile([128, NKT, NH_LOCAL, HD + 1], BF)  # natural + ones
            On = res.tile([128, 2, NQB, QB], BF)            # normalized attn out
            nc.gpsimd.memset(V[:, :, :, HD : HD + 1], 1.0)

            # input activations, 8 chunks of [128, S] each, in consumption order
            xq_sb = res.tile([128, NF, S], BF)
            xk_sb = res.tile([128, NF, S], BF)
            xv_sb = res.tile([128, NF, S], BF)
            for x_sb, x_ap in ((xq_sb, xq_ap), (xk_sb, xk_ap), (xv_sb, xv_ap)):
                for f in range(NF):
                    nc.sync.dma_start(x_sb[:, f, :], x_ap[f])

            # matmul psum outputs are limited to one bank (512 f32 cols)
            def mm512(out, lhsT, rhs, **kw):
                n = rhs.shape[-1]
                for j in range(0, n, 512):
                    w = min(512, n - j)
                    nc.tensor.matmul(
                        out[:, j : j + w], lhsT=lhsT, rhs=rhs[:, j : j + w],
                        **kw,
                    )

            def proj_pass(psP, name, x_sb, w_sb, b_sb, out_t, hfs, hook=None):
                """one (chan-half x q-half) quadrant group per entry in hfs"""
                pps = {}
                for hf, qh in hfs:
                    pps[(hf, qh)] = psP.tile(
                        [128, QB], F32, tag="pp", name=f"pp_{name}{hf}{qh}",
                    )
                for f in range(NF):
                    for hf, qh in hfs:
                        cs = slice(qh * QB, (qh + 1) * QB)
                        mm512(
                            pps[(hf, qh)],
                            lhsT=w_sb[:, f, hf * 128 : (hf + 1) * 128],
                            rhs=x_sb[:, f, cs],
                            start=(f == 0),
                            stop=(f == NF - 1),
                        )
                    if hook is not None:
                        hook(f)
                for hf, qh in hfs:
                    cs = slice(qh * QB, (qh + 1) * QB)
                    # on DVE, not Act: the scalar engine must stay free to
                    # run ahead on the attention exps
                    nc.vector.tensor_scalar_add(
                        out_t[:, hf, cs], pps[(hf, qh)][:],
                        b_sb[:, hf : hf + 1],
                    )

            ALL4 = [(hf, qh) for hf in range(2) for qh in range(2)]
            with tc.tile_pool(name="psA", bufs=4, space="PSUM") as psA:
                proj_pass(psA, "q", xq_sb, wq_sb, bq_sb, QT, ALL4)
                proj_pass(psA, "k", xk_sb, wk_sb, bk_sb, KT, ALL4)

            # --- attention + normalize + output projection, interleaved ---
            from contextlib import ExitStack as _ES
            stack_psO = _ES()
            with tc.tile_pool(name="psS", bufs=2, space="PSUM") as psB, stack_psO:
                units = [(h, qb) for qb in range(NQB) for h in range(NH_LOCAL)]

                def emit_scores(h, qb, kt, e_list):
                    rows = slice(64 * (h % 2), 64 * (h % 2) + 64)
                    s_t = psB.tile([128, QB], F32, tag="s",
                                   name=f"s_{h}{qb}{kt}")
                    mm512(
                        s_t,
                        lhsT=KT[rows, h // 2, kt * 128 : (kt + 1) * 128],
                        rhs=QT[rows, h // 2, qb * QB : (qb + 1) * QB],
                    )
                    e_t = work.tile([128, QB], BF, tag="e",
                                    name=f"e_{h}{qb}{kt}", bufs=12)
                    nc.scalar.activation(e_t[:], s_t[:], Exp, scale=0.125)
                    e_list.append(e_t)

                # V projection (two passes of two quadrants, 4 psum banks)
                # with unit (0,0) scores+exp prebuilds interleaved so the
                # scalar engine builds a deep lead before attention starts
                e00 = []
                with tc.tile_pool(name="psV", bufs=2, space="PSUM") as psV:
                    def hook1(f):
                        if f % 2 == 1:
                            emit_scores(0, 0, len(e00), e00)

                    proj_pass(psV, "v", xv_sb, wv_sb, bv_sb, VT,
                              [(0, 0), (0, 1)], hook=hook1)
                    proj_pass(psV, "v", xv_sb, wv_sb, bv_sb, VT,
                              [(1, 0), (1, 1)], hook=hook1)

                def transpose_tile(hf, tt):
                    tp = psB.tile([128, 128], BF, tag="s",
                                  name=f"tp_{hf}{tt}")
                    nc.tensor.transpose(
                        tp[:], VT[:, hf, tt * 128 : (tt + 1) * 128], id_sb[:]
                    )
                    nc.vector.tensor_copy(V[:, tt, 2 * hf, 0:HD], tp[:, 0:HD])
                    nc.vector.tensor_copy(
                        V[:, tt, 2 * hf + 1, 0:HD], tp[:, HD:128]
                    )

                # half0 transposes (heads 0,1) + keep prebuilding
                pending_tp = [(hf, tt) for hf in range(2) for tt in range(NKT)]
                for i in range(8):
                    transpose_tile(*pending_tp.pop(0))
                    transpose_tile(*pending_tp.pop(0))
                    if len(e00) < 12:
                        emit_scores(0, 0, len(e00), e00)

                psO = stack_psO.enter_context(
                    tc.tile_pool(name="psO", bufs=2, space="PSUM")
                )

                def normalize(h, qb):
                    """emit copy->shift->recip->broadcast->mul for unit"""
                    o_t, odd = o_tiles[(h, qb)]
                    rr = work.tile([HD + 1, QB], F32, tag="rr",
                                   name=f"rr_{h}{qb}", bufs=1)
                    nc.vector.tensor_copy(
                        rr[HD : HD + 1, :], o_t[HD : HD + 1, :]
                    )
                    rr0 = work.tile([1, QB], F32, tag="rr0",
                                    name=f"rr0_{h}{qb}", bufs=1)
                    nc.scalar.dma_start(rr0[:], rr[HD : HD + 1, :])
                    rrc = work.tile([1, QB], F32, tag="rrc",
                                    name=f"rrc_{h}{qb}", bufs=1)
                    nc.vector.reciprocal_approx_fast(rrc[:], rr0[:])
                    rbb = work.tile([HD, QB], F32, tag="rbb",
                                    name=f"rbb_{h}{qb}", bufs=1)
                    nc.gpsimd.partition_broadcast(rbb[:], rrc[:])
                    if not odd:
                        nc.vector.tensor_mul(
                            On[0:HD, h // 2, qb, :], o_t[0:HD, :], rbb[:]
                        )
                    else:
                        ot = work.tile([HD, QB], BF, tag="ot",
                                       name=f"ot_{h}{qb}", bufs=1)
                        nc.vector.tensor_mul(ot[:], o_t[0:HD, :], rbb[:])
                        nc.scalar.dma_start(On[HD:128, h // 2, qb, :], ot[:])

                def outproj_step(qb, oc):
                    py = psB.tile([128, QB], F32, tag="s", name=f"py_{qb}{oc}")
                    for hf in range(2):
                        mm512(
                            py,
                            lhsT=wo_sb[:, hf, oc, :],
                            rhs=On[:, hf, qb, :],
                            start=(hf == 0),
                            stop=(hf == 1),
                        )
                    ysb = work.tile([128, QB], BF, tag="y", name=f"y_{qb}{oc}",
                                    bufs=4)
                    nc.vector.tensor_copy(ysb[:], py[:])
                    eng = (nc.sync, nc.scalar, nc.gpsimd)[oc % 3]
                    eng.dma_start(
                        y_ap[oc, :, qb * QB : (qb + 1) * QB], ysb[:]
                    )

                o_tiles = {}
                pending_norm = []
                pending_out = []
                for h, qb in units:
                    o_t = psO.tile([HD + 1, QB], F32, tag="o", name=f"o_{h}{qb}")
                    o_tiles[(h, qb)] = (o_t, h % 2 == 1)
                    e_tiles = e00 if (h, qb) == (0, 0) else []
                    for kt in range(NKT + 1):
                        if kt < NKT and len(e_tiles) <= kt:
                            emit_scores(h, qb, kt, e_tiles)
                        # interleave deferred work into the PE stream
                        if kt == 2 and pending_norm:
                            normalize(*pending_norm.pop(0))
                        if kt in (3, 5, 7, 9, 11, 13, 14, 15) and pending_out:
                            outproj_step(*pending_out.pop(0))
                        elif kt % 2 == 1 and pending_tp:
                            transpose_tile(*pending_tp.pop(0))
                        if kt >= 1:
                            k0 = kt - 1
                            mm512(
                                o_t,
                                lhsT=V[:, k0, h, :],
                                rhs=e_tiles[k0][:],
                                start=(k0 == 0),
                                stop=(k0 == NKT - 1),
                            )
                    pending_norm.append((h, qb))
                    if h == NH_LOCAL - 1:
                        pending_out.extend((qb, oc) for oc in range(NF))
                # drain
                while pending_norm:
                    normalize(*pending_norm.pop(0))
                while pending_out:
                    outproj_step(*pending_out.pop(0))
    nc.compile()
    return nc


def _get_nc():
    global _nc
    with _cache:
        if _nc is None:
            _nc = _build_nc()
        return _nc


def kernel(q, k, v, wq_w, wq_b, wk_w, wk_b, wv_w, wv_b, wo_w, wo_b):
    global LAST_RESULT
    nc = _get_nc()

    q = np.asarray(q, dtype=np.float32)
    k = np.asarray(k, dtype=np.float32)
    v = np.asarray(v, dtype=np.float32)
    wq_w = np.asarray(wq_w, dtype=np.float32)
    wk_w = np.asarray(wk_w, dtype=np.float32)
    wv_w = np.asarray(wv_w, dtype=np.float32)
    wo_w = np.asarray(wo_w, dtype=np.float32)

    def xT(a, b):
        return np.ascontiguousarray(a[b].astype(BF16).T)

    def b2(a, cs):
        return np.ascontiguousarray(
            np.asarray(a, np.float32)[cs].reshape(2, 128).T
        )

    ident = np.eye(128, dtype=BF16)

    in_maps = []
    for c in range(N_CORES):
        b = c // 4
        hg = c % 4
        cs = slice(hg * CPC, (hg + 1) * CPC)
        in_maps.append({
            "xq_t": xT(q, b),
            "xk_t": xT(k, b),
            "xv_t": xT(v, b),
            "wq_t": np.ascontiguousarray(wq_w[cs, :].astype(BF16).T),
            "wk_t": np.ascontiguousarray(wk_w[cs, :].astype(BF16).T),
            "wv_t": np.ascontiguousarray(wv_w[cs, :].astype(BF16).T),
            "bq": b2(wq_b, cs),
            "bk": b2(wk_b, cs),
            "bv": b2(wv_b, cs),
            "wo_t": np.ascontiguousarray(wo_w[:, cs].astype(BF16).T),
            "ident": ident,
        })

    res = run_bass_kernel_spmd(
        nc, in_maps, core_ids=list(range(N_CORES)),
        trace=bool(int(os.environ.get("MHA_TRACE", "0"))),
    )
    LAST_RESULT = res

    ys = []
    for b in range(B):
        y = res.results[b * 4]["y_t"].astype(np.float64)
        for hg in range(1, 4):
            y += res.results[b * 4 + hg]["y_t"]
        ys.append(y.T)
    y = np.stack(ys) + np.asarray(wo_b, np.float64)[None, None, :]
    return y.astype(np.float32)


# revision 34
# speedup vs baseline: 1.1050x; 1.0718x over previous
"""Multi-head attention (B=2, S=2048, H=1024, 16 heads) on 8 trn2 NeuronCores.

Sharding: 2-way batch x 4-way head-group tensor parallel. Core c handles
batch c//4 and heads 4*(c%4)..4*(c%4)+3 (256 channels of the QKV
projections / 256 input channels of the output projection). Each core
consumes only its batch's activations (halves HBM traffic vs full
replication); the 4 partial wo outputs per batch are summed on the host.

Device-side dataflow per core (bf16 matmuls, f32 PSUM):
  QT/KT[c, s]   : transposed projections, channels on partitions
  VT[c, s] -> V : PE-transposed to natural layout, ones-augmented (65 cols)
  S^T[k, q]     = KT_h^T-tile . QT_h              (per head, 128-key tiles)
  E = exp(S/8)  (no max subtraction: scores ~ N(0,1))
  O^T[65, q]    accumulates V_aug^T . E over 16 key tiles (row 64 = sums)
  r = 1/sums    on one partition row; broadcast via a K=1 matmul
  On = O * r    ; y^T partial = wo_c^T . On, DMA'd straight from PSUM

The emission interleaves the second half of the projections, the
normalization matmuls and the output projection into the attention
stream so the PE never idles (idle gaps drop it out of max p-state).
"""

import os
import threading

import numpy as np
import ml_dtypes

import concourse.bass as bass
import concourse.mybir as mybir
import concourse.tile as tile
from concourse import bacc
from concourse.bass_utils import run_bass_kernel_spmd

BF16 = ml_dtypes.bfloat16
F32 = mybir.dt.float32
BF = mybir.dt.bfloat16

B = 2
S = 2048
H = 1024
NH_LOCAL = 4        # heads per core
HD = 64             # head dim
CPC = 256           # channels per core
NF = H // 128       # contraction chunks
NKT = S // 128      # key tiles
NQB = 2             # q blocks of 1024
QB = S // NQB
N_CORES = 8

_cache = threading.Lock()
_nc = None

LAST_RESULT = None  # BassKernelResults of the most recent run (for test.py)


def _build_nc():
    nc = bacc.Bacc(None, target_bir_lowering=False, debug=False)

    xq_d = nc.dram_tensor("xq_t", [H, S], BF, kind="ExternalInput")
    xk_d = nc.dram_tensor("xk_t", [H, S], BF, kind="ExternalInput")
    xv_d = nc.dram_tensor("xv_t", [H, S], BF, kind="ExternalInput")
    wq_d = nc.dram_tensor("wq_t", [H, CPC], BF, kind="ExternalInput")
    wk_d = nc.dram_tensor("wk_t", [H, CPC], BF, kind="ExternalInput")
    wv_d = nc.dram_tensor("wv_t", [H, CPC], BF, kind="ExternalInput")
    bq_d = nc.dram_tensor("bq", [128, 2], F32, kind="ExternalInput")
    bk_d = nc.dram_tensor("bk", [128, 2], F32, kind="ExternalInput")
    bv_d = nc.dram_tensor("bv", [128, 2], F32, kind="ExternalInput")
    wo_d = nc.dram_tensor("wo_t", [CPC, H], BF, kind="ExternalInput")
    id_d = nc.dram_tensor("ident", [128, 128], BF, kind="ExternalInput")
    y_d = nc.dram_tensor("y_t", [H, S], BF, kind="ExternalOutput")

    xq_ap = xq_d.rearrange("(f p) s -> f p s", p=128)
    xk_ap = xk_d.rearrange("(f p) s -> f p s", p=128)
    xv_ap = xv_d.rearrange("(f p) s -> f p s", p=128)
    y_ap = y_d.rearrange("(oc p) s -> oc p s", p=128)

    Exp = mybir.ActivationFunctionType.Exp
    Copy = mybir.ActivationFunctionType.Identity

    with tile.TileContext(nc) as tc:
        with (
            tc.tile_pool(name="const", bufs=1) as const,
            tc.tile_pool(name="res", bufs=1) as res,
            tc.tile_pool(name="work", bufs=4) as work,
        ):
            # --- constants / weights ---
            wq_sb = const.tile([128, NF, CPC], BF)
            wk_sb = const.tile([128, NF, CPC], BF)
            wv_sb = const.tile([128, NF, CPC], BF)
            wo_sb = const.tile([128, 2, NF, 128], BF)
            bq_sb = const.tile([128, 2], F32)
            bk_sb = const.tile([128, 2], F32)
            bv_sb = const.tile([128, 2], F32)
            id_sb = const.tile([128, 128], BF)
            nc.sync.dma_start(wq_sb[:], wq_d.rearrange("(f p) c -> p f c", p=128))
            nc.sync.dma_start(wk_sb[:], wk_d.rearrange("(f p) c -> p f c", p=128))
            nc.sync.dma_start(wv_sb[:], wv_d.rearrange("(f p) c -> p f c", p=128))
            nc.sync.dma_start(wo_sb[:], wo_d.rearrange("(hf p) (oc c) -> p hf oc c", p=128, c=128))
            nc.sync.dma_start(bq_sb[:], bq_d[:])
            nc.sync.dma_start(bk_sb[:], bk_d[:])
            nc.sync.dma_start(bv_sb[:], bv_d[:])
            nc.sync.dma_start(id_sb[:], id_d[:])

            # --- residents ---
            QT = res.tile([128, 2, S], BF)     # [p, chan-half, tok]
            KT = res.tile([128, 2, S], BF)
            VT = res.tile([128, 2, S], BF)
            V = res.tile([128, NKT, NH_LOCAL, HD + 1], BF)  # natural + ones
            On = res.tile([128, 2, NQB, QB], BF)            # normalized attn out
            nc.gpsimd.memset(V[:, :, :, HD : HD + 1], 1.0)

            # input activations, 8 chunks of [128, S] each, in consumption order
            xq_sb = res.tile([128, NF, S], BF)
            xk_sb = res.tile([128, NF, S], BF)
            xv_sb = res.tile([128, NF, S], BF)
            for x_sb, x_ap in ((xq_sb, xq_ap), (xk_sb, xk_ap), (xv_sb, xv_ap)):
                for f in range(NF):
                    nc.sync.dma_start(x_sb[:, f, :], x_ap[f])

            # matmul psum outputs are limited to one bank (512 f32 cols)
            def mm512(out, lhsT, rhs, **kw):
                n = rhs.shape[-1]
                for j in range(0, n, 512):
                    w = min(512, n - j)
                    nc.tensor.matmul(
                        out[:, j : j + w], lhsT=lhsT, rhs=rhs[:, j : j + w],
                        **kw,
                    )

            def proj_pass(psP, name, x_sb, w_sb, b_sb, out_t, hfs, hook=None):
                """one (chan-half x q-half) quadrant group per entry in hfs"""
                pps = {}
                for hf, qh in hfs:
                    pps[(hf, qh)] = psP.tile(
                        [128, QB], F32, tag="pp", name=f"pp_{name}{hf}{qh}",
                    )
                for f in range(NF):
                    for hf, qh in hfs:
                        cs = slice(qh * QB, (qh + 1) * QB)
                        mm512(
                            pps[(hf, qh)],
                            lhsT=w_sb[:, f, hf * 128 : (hf + 1) * 128],
                            rhs=x_sb[:, f, cs],
                            start=(f == 0),
                            stop=(f == NF - 1),
                        )
                    if hook is not None:
                        hook(f)
                for hf, qh in hfs:
                    cs = slice(qh * QB, (qh + 1) * QB)
                    # on DVE, not Act: the scalar engine must stay free to
                    # run ahead on the attention exps
                    nc.vector.tensor_scalar_add(
                        out_t[:, hf, cs], pps[(hf, qh)][:],
                        b_sb[:, hf : hf + 1],
                    )

            ALL4 = [(hf, qh) for hf in range(2) for qh in range(2)]
            with tc.tile_pool(name="psA", bufs=4, space="PSUM") as psA:
                proj_pass(psA, "q", xq_sb, wq_sb, bq_sb, QT, ALL4)
                proj_pass(psA, "k", xk_sb, wk_sb, bk_sb, KT, ALL4)

            # --- attention + normalize + output projection, interleaved ---
            from contextlib import ExitStack as _ES
            stack_psO = _ES()
            with tc.tile_pool(name="psS", bufs=2, space="PSUM") as psB, stack_psO:
                units = [(h, qb) for qb in range(NQB) for h in range(NH_LOCAL)]

                def emit_scores(h, qb, kt, e_list):
                    rows = slice(64 * (h % 2), 64 * (h % 2) + 64)
                    s_t = psB.tile([128, QB], F32, tag="s",
                                   name=f"s_{h}{qb}{kt}")
                    mm512(
                        s_t,
                        lhsT=KT[rows, h // 2, kt * 128 : (kt + 1) * 128],
                        rhs=QT[rows, h // 2, qb * QB : (qb + 1) * QB],
                    )
                    e_t = work.tile([128, QB], BF, tag="e",
                                    name=f"e_{h}{qb}{kt}", bufs=12)
                    nc.scalar.activation(e_t[:], s_t[:], Exp, scale=0.125)
                    e_list.append(e_t)

                # V projection (two passes of two quadrants, 4 psum banks)
                # with unit (0,0) scores+exp prebuilds interleaved so the
                # scalar engine builds a deep lead before attention starts
                e00 = []
                with tc.tile_pool(name="psV", bufs=2, space="PSUM") as psV:
                    def hook1(f):
                        if f % 2 == 1:
                            emit_scores(0, 0, len(e00), e00)

                    proj_pass(psV, "v", xv_sb, wv_sb, bv_sb, VT,
                              [(0, 0), (0, 1)], hook=hook1)
                    proj_pass(psV, "v", xv_sb, wv_sb, bv_sb, VT,
                              [(1, 0), (1, 1)], hook=hook1)

                def transpose_tile(hf, tt):
                    tp = psB.tile([128, 128], BF, tag="s",
                                  name=f"tp_{hf}{tt}")
                    nc.tensor.transpose(
                        tp[:], VT[:, hf, tt * 128 : (tt + 1) * 128], id_sb[:]
                    )
                    nc.vector.tensor_copy(V[:, tt, 2 * hf, 0:HD], tp[:, 0:HD])
                    nc.vector.tensor_copy(
                        V[:, tt, 2 * hf + 1, 0:HD], tp[:, HD:128]
                    )

                # half0 transposes (heads 0,1) + keep prebuilding
                pending_tp = [(hf, tt) for hf in range(2) for tt in range(NKT)]
                for i in range(8):
                    transpose_tile(*pending_tp.pop(0))
                    transpose_tile(*pending_tp.pop(0))
                    if len(e00) < 12:
                        emit_scores(0, 0, len(e00), e00)

                psO = stack_psO.enter_context(
                    tc.tile_pool(name="psO", bufs=2, space="PSUM")
                )

                def normalize(h, qb):
                    """emit copy->shift->recip->broadcast->mul for unit"""
                    o_t, odd = o_tiles[(h, qb)]
                    rr = work.tile([HD + 1, QB], F32, tag="rr",
                                   name=f"rr_{h}{qb}", bufs=1)
                    nc.vector.tensor_copy(
                        rr[HD : HD + 1, :], o_t[HD : HD + 1, :]
                    )
                    rr0 = work.tile([1, QB], F32, tag="rr0",
                                    name=f"rr0_{h}{qb}", bufs=1)
                    nc.scalar.dma_start(rr0[:], rr[HD : HD + 1, :])
                    rrc = work.tile([1, QB], F32, tag="rrc",
                                    name=f"rrc_{h}{qb}", bufs=1)
                    nc.vector.reciprocal_approx_fast(rrc[:], rr0[:])
                    rbb = work.tile([HD, QB], F32, tag="rbb",
                                    name=f"rbb_{h}{qb}", bufs=1)
                    nc.gpsimd.partition_broadcast(rbb[:], rrc[:])
                    if not odd:
                        nc.vector.tensor_mul(
                            On[0:HD, h // 2, qb, :], o_t[0:HD, :], rbb[:]
                        )
                    else:
                        ot = work.tile([HD, QB], BF, tag="ot",
                                       name=f"ot_{h}{qb}", bufs=1)
                        nc.vector.tensor_mul(ot[:], o_t[0:HD, :], rbb[:])
                        nc.scalar.dma_start(On[HD:128, h // 2, qb, :], ot[:])

                def outproj_step(qb, oc):
                    py = psB.tile([128, QB], F32, tag="s", name=f"py_{qb}{oc}")
                    for hf in range(2):
                        mm512(
                            py,
                            lhsT=wo_sb[:, hf, oc, :],
                            rhs=On[:, hf, qb, :],
                            start=(hf == 0),
                            stop=(hf == 1),
                        )
                    ysb = work.tile([128, QB], BF, tag="y", name=f"y_{qb}{oc}",
                                    bufs=4)
                    nc.vector.tensor_copy(ysb[:], py[:])
                    eng = (nc.sync, nc.scalar, nc.gpsimd)[oc % 3]
                    eng.dma_start(
                        y_ap[oc, :, qb * QB : (qb + 1) * QB], ysb[:]
                    )

                o_tiles = {}
                pending_norm = []
                pending_out = []
                LAG = 3  # AV(k) issues ~3 slots after S(k): covers exp latency
                for h, qb in units:
                    o_t = psO.tile([HD + 1, QB], F32, tag="o", name=f"o_{h}{qb}")
                    o_tiles[(h, qb)] = (o_t, h % 2 == 1)
                    e_tiles = e00 if (h, qb) == (0, 0) else []
                    for kt in range(NKT + LAG):
                        if kt < NKT and len(e_tiles) <= kt:
                            emit_scores(h, qb, kt, e_tiles)
                        # interleave deferred work into the PE stream
                        if kt == 2 and pending_norm:
                            normalize(*pending_norm.pop(0))
                        if kt in (3, 5, 7, 9, 11, 13, 14, 15) and pending_out:
                            outproj_step(*pending_out.pop(0))
                        elif kt % 2 == 1 and pending_tp:
                            transpose_tile(*pending_tp.pop(0))
                        if kt >= LAG:
                            k0 = kt - LAG
                            mm512(
                                o_t,
                                lhsT=V[:, k0, h, :],
                                rhs=e_tiles[k0][:],
                                start=(k0 == 0),
                                stop=(k0 == NKT - 1),
                            )
                    pending_norm.append((h, qb))
                    if h == NH_LOCAL - 1:
                        pending_out.extend((qb, oc) for oc in range(NF))
                # drain
                while pending_norm:
                    normalize(*pending_norm.pop(0))
                while pending_out:
                    outproj_step(*pending_out.pop(0))
    nc.compile()
    return nc


def _get_nc():
    global _nc
    with _cache:
        if _nc is None:
            _nc = _build_nc()
        return _nc


def kernel(q, k, v, wq_w, wq_b, wk_w, wk_b, wv_w, wv_b, wo_w, wo_b):
    global LAST_RESULT
    nc = _get_nc()

    q = np.asarray(q, dtype=np.float32)
    k = np.asarray(k, dtype=np.float32)
    v = np.asarray(v, dtype=np.float32)
    wq_w = np.asarray(wq_w, dtype=np.float32)
    wk_w = np.asarray(wk_w, dtype=np.float32)
    wv_w = np.asarray(wv_w, dtype=np.float32)
    wo_w = np.asarray(wo_w, dtype=np.float32)

    def xT(a, b):
        return np.ascontiguousarray(a[b].astype(BF16).T)

    def b2(a, cs):
        return np.ascontiguousarray(
            np.asarray(a, np.float32)[cs].reshape(2, 128).T
        )

    ident = np.eye(128, dtype=BF16)

    in_maps = []
    for c in range(N_CORES):
        b = c // 4
        hg = c % 4
        cs = slice(hg * CPC, (hg + 1) * CPC)
        in_maps.append({
            "xq_t": xT(q, b),
            "xk_t": xT(k, b),
            "xv_t": xT(v, b),
            "wq_t": np.ascontiguousarray(wq_w[cs, :].astype(BF16).T),
            "wk_t": np.ascontiguousarray(wk_w[cs, :].astype(BF16).T),
            "wv_t": np.ascontiguousarray(wv_w[cs, :].astype(BF16).T),
            "bq": b2(wq_b, cs),
            "bk": b2(wk_b, cs),
            "bv": b2(wv_b, cs),
            "wo_t": np.ascontiguousarray(wo_w[:, cs].astype(BF16).T),
            "ident": ident,
        })

    res = run_bass_kernel_spmd(
        nc, in_maps, core_ids=list(range(N_CORES)),
        trace=bool(int(os.environ.get("MHA_TRACE", "0"))),
    )
    LAST_RESULT = res

    ys = []
    for b in range(B):
        y = res.results[b * 4]["y_t"].astype(np.float64)
        for hg in range(1, 4):
            y += res.results[b * 4 + hg]["y_t"]
        ys.append(y.T)
    y = np.stack(ys) + np.asarray(wo_b, np.float64)[None, None, :]
    return y.astype(np.float32)


# revision 35
# speedup vs baseline: 1.1432x; 1.0346x over previous
"""Multi-head attention (B=2, S=2048, H=1024, 16 heads) on 8 trn2 NeuronCores.

Sharding: 2-way batch x 4-way head-group tensor parallel. Core c handles
batch c//4 and heads 4*(c%4)..4*(c%4)+3 (256 channels of the QKV
projections / 256 input channels of the output projection). Each core
consumes only its batch's activations (halves HBM traffic vs full
replication); the 4 partial wo outputs per batch are summed on the host.

Device-side dataflow per core (bf16 matmuls, f32 PSUM):
  QT/KT[c, s]   : transposed projections, channels on partitions
  VT[c, s] -> V : PE-transposed to natural layout, ones-augmented (65 cols)
  S^T[k, q]     = KT_h^T-tile . QT_h              (per head, 128-key tiles)
  E = exp(S/8)  (no max subtraction: scores ~ N(0,1))
  O^T[65, q]    accumulates V_aug^T . E over 16 key tiles (row 64 = sums)
  r = 1/sums    on one partition row; broadcast via a K=1 matmul
  On = O * r    ; y^T partial = wo_c^T . On, DMA'd straight from PSUM

The emission interleaves the second half of the projections, the
normalization matmuls and the output projection into the attention
stream so the PE never idles (idle gaps drop it out of max p-state).
"""

import os
import threading

import numpy as np
import ml_dtypes

import concourse.bass as bass
import concourse.mybir as mybir
import concourse.tile as tile
from concourse import bacc
from concourse.bass_utils import run_bass_kernel_spmd

BF16 = ml_dtypes.bfloat16
F32 = mybir.dt.float32
BF = mybir.dt.bfloat16

B = 2
S = 2048
H = 1024
NH_LOCAL = 4        # heads per core
HD = 64             # head dim
CPC = 256           # channels per core
NF = H // 128       # contraction chunks
NKT = S // 128      # key tiles
NQB = 2             # q blocks of 1024
QB = S // NQB
N_CORES = 8

_cache = threading.Lock()
_nc = None

LAST_RESULT = None  # BassKernelResults of the most recent run (for test.py)


def _build_nc():
    nc = bacc.Bacc(None, target_bir_lowering=False, debug=False)

    xq_d = nc.dram_tensor("xq_t", [H, S], BF, kind="ExternalInput")
    xk_d = nc.dram_tensor("xk_t", [H, S], BF, kind="ExternalInput")
    xv_d = nc.dram_tensor("xv_t", [H, S], BF, kind="ExternalInput")
    wq_d = nc.dram_tensor("wq_t", [H, CPC], BF, kind="ExternalInput")
    wk_d = nc.dram_tensor("wk_t", [H, CPC], BF, kind="ExternalInput")
    wv_d = nc.dram_tensor("wv_t", [H, CPC], BF, kind="ExternalInput")
    bq_d = nc.dram_tensor("bq", [128, 2], F32, kind="ExternalInput")
    bk_d = nc.dram_tensor("bk", [128, 2], F32, kind="ExternalInput")
    bv_d = nc.dram_tensor("bv", [128, 2], F32, kind="ExternalInput")
    wo_d = nc.dram_tensor("wo_t", [CPC, H], BF, kind="ExternalInput")
    id_d = nc.dram_tensor("ident", [128, 128], BF, kind="ExternalInput")
    y_d = nc.dram_tensor("y_t", [H, S], BF, kind="ExternalOutput")

    xq_ap = xq_d.rearrange("(f p) s -> f p s", p=128)
    xk_ap = xk_d.rearrange("(f p) s -> f p s", p=128)
    xv_ap = xv_d.rearrange("(f p) s -> f p s", p=128)
    y_ap = y_d.rearrange("(oc p) s -> oc p s", p=128)

    Exp = mybir.ActivationFunctionType.Exp
    Copy = mybir.ActivationFunctionType.Identity

    with tile.TileContext(nc) as tc:
        with (
            tc.tile_pool(name="const", bufs=1) as const,
            tc.tile_pool(name="res", bufs=1) as res,
            tc.tile_pool(name="work", bufs=4) as work,
        ):
            # --- constants / weights ---
            wq_sb = const.tile([128, NF, CPC], BF)
            wk_sb = const.tile([128, NF, CPC], BF)
            wv_sb = const.tile([128, NF, CPC], BF)
            wo_sb = const.tile([128, 2, NF, 128], BF)
            bq_sb = const.tile([128, 2], F32)
            bk_sb = const.tile([128, 2], F32)
            bv_sb = const.tile([128, 2], F32)
            id_sb = const.tile([128, 128], BF)
            nc.sync.dma_start(wq_sb[:], wq_d.rearrange("(f p) c -> p f c", p=128))
            nc.sync.dma_start(wk_sb[:], wk_d.rearrange("(f p) c -> p f c", p=128))
            nc.sync.dma_start(wv_sb[:], wv_d.rearrange("(f p) c -> p f c", p=128))
            nc.sync.dma_start(wo_sb[:], wo_d.rearrange("(hf p) (oc c) -> p hf oc c", p=128, c=128))
            nc.sync.dma_start(bq_sb[:], bq_d[:])
            nc.sync.dma_start(bk_sb[:], bk_d[:])
            nc.sync.dma_start(bv_sb[:], bv_d[:])
            nc.sync.dma_start(id_sb[:], id_d[:])

            # --- residents ---
            QT = res.tile([128, 2, S], BF)     # [p, chan-half, tok]
            KT = res.tile([128, 2, S], BF)
            VT = res.tile([128, 2, S], BF)
            V = res.tile([128, NKT, NH_LOCAL, HD + 1], BF)  # natural + ones
            On = res.tile([128, 2, NQB, QB], BF)            # normalized attn out
            nc.gpsimd.memset(V[:, :, :, HD : HD + 1], 1.0)

            # input activations, 8 chunks of [128, S] each, in consumption order
            xq_sb = res.tile([128, NF, S], BF)
            xk_sb = res.tile([128, NF, S], BF)
            xv_sb = res.tile([128, NF, S], BF)
            for x_sb, x_ap in ((xq_sb, xq_ap), (xk_sb, xk_ap), (xv_sb, xv_ap)):
                for f in range(NF):
                    nc.sync.dma_start(x_sb[:, f, :], x_ap[f])

            # matmul psum outputs are limited to one bank (512 f32 cols)
            def mm512(out, lhsT, rhs, **kw):
                n = rhs.shape[-1]
                for j in range(0, n, 512):
                    w = min(512, n - j)
                    nc.tensor.matmul(
                        out[:, j : j + w], lhsT=lhsT, rhs=rhs[:, j : j + w],
                        **kw,
                    )

            def proj_pass(psP, name, x_sb, w_sb, b_sb, out_t, hfs, hook=None):
                """one (chan-half x q-half) quadrant group per entry in hfs"""
                pps = {}
                for hf, qh in hfs:
                    pps[(hf, qh)] = psP.tile(
                        [128, QB], F32, tag="pp", name=f"pp_{name}{hf}{qh}",
                    )
                for f in range(NF):
                    for hf, qh in hfs:
                        cs = slice(qh * QB, (qh + 1) * QB)
                        mm512(
                            pps[(hf, qh)],
                            lhsT=w_sb[:, f, hf * 128 : (hf + 1) * 128],
                            rhs=x_sb[:, f, cs],
                            start=(f == 0),
                            stop=(f == NF - 1),
                        )
                    if hook is not None:
                        hook(f)
                for hf, qh in hfs:
                    cs = slice(qh * QB, (qh + 1) * QB)
                    # on DVE, not Act: the scalar engine must stay free to
                    # run ahead on the attention exps
                    nc.vector.tensor_scalar_add(
                        out_t[:, hf, cs], pps[(hf, qh)][:],
                        b_sb[:, hf : hf + 1],
                    )

            ALL4 = [(hf, qh) for hf in range(2) for qh in range(2)]
            with tc.tile_pool(name="psA", bufs=4, space="PSUM") as psA:
                proj_pass(psA, "q", xq_sb, wq_sb, bq_sb, QT, ALL4)
                proj_pass(psA, "k", xk_sb, wk_sb, bk_sb, KT, ALL4)

            # --- attention + normalize + output projection, interleaved ---
            from contextlib import ExitStack as _ES
            stack_psO = _ES()
            with tc.tile_pool(name="psS", bufs=2, space="PSUM") as psB, stack_psO:
                units = [(h, qb) for qb in range(NQB) for h in range(NH_LOCAL)]

                def emit_scores(h, qb, kt, e_list):
                    rows = slice(64 * (h % 2), 64 * (h % 2) + 64)
                    s_t = psB.tile([128, QB], F32, tag="s",
                                   name=f"s_{h}{qb}{kt}")
                    mm512(
                        s_t,
                        lhsT=KT[rows, h // 2, kt * 128 : (kt + 1) * 128],
                        rhs=QT[rows, h // 2, qb * QB : (qb + 1) * QB],
                    )
                    e_t = work.tile([128, QB], BF, tag="e",
                                    name=f"e_{h}{qb}{kt}", bufs=12)
                    nc.scalar.activation(e_t[:], s_t[:], Exp, scale=0.125)
                    e_list.append(e_t)

                # V projection (two passes of two quadrants, 4 psum banks)
                # with unit (0,0) scores+exp prebuilds interleaved so the
                # scalar engine builds a deep lead before attention starts
                e00 = []
                with tc.tile_pool(name="psV", bufs=2, space="PSUM") as psV:
                    def hook1(f):
                        if f % 2 == 1:
                            emit_scores(0, 0, len(e00), e00)

                    proj_pass(psV, "v", xv_sb, wv_sb, bv_sb, VT,
                              [(0, 0), (0, 1)], hook=hook1)
                    proj_pass(psV, "v", xv_sb, wv_sb, bv_sb, VT,
                              [(1, 0), (1, 1)], hook=hook1)

                def transpose_tile(hf, tt):
                    tp = psB.tile([128, 128], BF, tag="s",
                                  name=f"tp_{hf}{tt}")
                    nc.tensor.transpose(
                        tp[:], VT[:, hf, tt * 128 : (tt + 1) * 128], id_sb[:]
                    )
                    nc.vector.tensor_copy(V[:, tt, 2 * hf, 0:HD], tp[:, 0:HD])
                    nc.vector.tensor_copy(
                        V[:, tt, 2 * hf + 1, 0:HD], tp[:, HD:128]
                    )

                # half0 transposes (heads 0,1) + keep prebuilding
                pending_tp = [(hf, tt) for hf in range(2) for tt in range(NKT)]
                for i in range(8):
                    transpose_tile(*pending_tp.pop(0))
                    transpose_tile(*pending_tp.pop(0))
                    if len(e00) < 12:
                        emit_scores(0, 0, len(e00), e00)

                psO = stack_psO.enter_context(
                    tc.tile_pool(name="psO", bufs=2, space="PSUM")
                )

                def normalize(h, qb):
                    """emit copy->shift->recip->broadcast->mul for unit"""
                    o_t, odd = o_tiles[(h, qb)]
                    rr = work.tile([HD + 1, QB], F32, tag="rr",
                                   name=f"rr_{h}{qb}", bufs=1)
                    nc.vector.tensor_copy(
                        rr[HD : HD + 1, :], o_t[HD : HD + 1, :]
                    )
                    rr0 = work.tile([1, QB], F32, tag="rr0",
                                    name=f"rr0_{h}{qb}", bufs=1)
                    nc.scalar.dma_start(rr0[:], rr[HD : HD + 1, :])
                    rrc = work.tile([1, QB], F32, tag="rrc",
                                    name=f"rrc_{h}{qb}", bufs=1)
                    nc.vector.reciprocal_approx_fast(rrc[:], rr0[:])
                    rbb = work.tile([HD, QB], F32, tag="rbb",
                                    name=f"rbb_{h}{qb}", bufs=1)
                    nc.gpsimd.partition_broadcast(rbb[:], rrc[:])
                    if not odd:
                        nc.vector.tensor_mul(
                            On[0:HD, h // 2, qb, :], o_t[0:HD, :], rbb[:]
                        )
                    else:
                        ot = work.tile([HD, QB], BF, tag="ot",
                                       name=f"ot_{h}{qb}", bufs=1)
                        nc.vector.tensor_mul(ot[:], o_t[0:HD, :], rbb[:])
                        nc.scalar.dma_start(On[HD:128, h // 2, qb, :], ot[:])

                def outproj_step(qb, oc):
                    py = psB.tile([128, QB], F32, tag="s", name=f"py_{qb}{oc}")
                    for hf in range(2):
                        mm512(
                            py,
                            lhsT=wo_sb[:, hf, oc, :],
                            rhs=On[:, hf, qb, :],
                            start=(hf == 0),
                            stop=(hf == 1),
                        )
                    ysb = work.tile([128, QB], BF, tag="y", name=f"y_{qb}{oc}",
                                    bufs=4)
                    nc.vector.tensor_copy(ysb[:], py[:])
                    eng = (nc.sync, nc.scalar, nc.gpsimd)[oc % 3]
                    eng.dma_start(
                        y_ap[oc, :, qb * QB : (qb + 1) * QB], ysb[:]
                    )

                o_tiles = {}
                pending_norm = []
                pending_out = []
                LAG = 6  # AV(k) issues ~6 slots after S(k): deep exp cushion
                for h, qb in units:
                    o_t = psO.tile([HD + 1, QB], F32, tag="o", name=f"o_{h}{qb}")
                    o_tiles[(h, qb)] = (o_t, h % 2 == 1)
                    e_tiles = e00 if (h, qb) == (0, 0) else []
                    for kt in range(NKT + LAG):
                        if kt < NKT and len(e_tiles) <= kt:
                            emit_scores(h, qb, kt, e_tiles)
                        # interleave deferred work into the PE stream
                        if kt == 2 and pending_norm:
                            normalize(*pending_norm.pop(0))
                        if kt in (5, 11) and pending_out:
                            outproj_step(*pending_out.pop(0))
                        elif kt % 2 == 1 and pending_tp:
                            transpose_tile(*pending_tp.pop(0))
                        if kt >= LAG:
                            k0 = kt - LAG
                            mm512(
                                o_t,
                                lhsT=V[:, k0, h, :],
                                rhs=e_tiles[k0][:],
                                start=(k0 == 0),
                                stop=(k0 == NKT - 1),
                            )
                    pending_norm.append((h, qb))
                    if h == NH_LOCAL - 1:
                        pending_out.extend((qb, oc) for oc in range(NF))
                # drain
                while pending_norm:
                    normalize(*pending_norm.pop(0))
                while pending_out:
                    outproj_step(*pending_out.pop(0))
    nc.compile()
    return nc


def _get_nc():
    global _nc
    with _cache:
        if _nc is None:
            _nc = _build_nc()
        return _nc


def kernel(q, k, v, wq_w, wq_b, wk_w, wk_b, wv_w, wv_b, wo_w, wo_b):
    global LAST_RESULT
    nc = _get_nc()

    q = np.asarray(q, dtype=np.float32)
    k = np.asarray(k, dtype=np.float32)
    v = np.asarray(v, dtype=np.float32)
    wq_w = np.asarray(wq_w, dtype=np.float32)
    wk_w = np.asarray(wk_w, dtype=np.float32)
    wv_w = np.asarray(wv_w, dtype=np.float32)
    wo_w = np.asarray(wo_w, dtype=np.float32)

    def xT(a, b):
        return np.ascontiguousarray(a[b].astype(BF16).T)

    def b2(a, cs):
        return np.ascontiguousarray(
            np.asarray(a, np.float32)[cs].reshape(2, 128).T
        )

    ident = np.eye(128, dtype=BF16)

    in_maps = []
    for c in range(N_CORES):
        b = c // 4
        hg = c % 4
        cs = slice(hg * CPC, (hg + 1) * CPC)
        in_maps.append({
            "xq_t": xT(q, b),
            "xk_t": xT(k, b),
            "xv_t": xT(v, b),
            "wq_t": np.ascontiguousarray(wq_w[cs, :].astype(BF16).T),
            "wk_t": np.ascontiguousarray(wk_w[cs, :].astype(BF16).T),
            "wv_t": np.ascontiguousarray(wv_w[cs, :].astype(BF16).T),
            "bq": b2(wq_b, cs),
            "bk": b2(wk_b, cs),
            "bv": b2(wv_b, cs),
            "wo_t": np.ascontiguousarray(wo_w[:, cs].astype(BF16).T),
            "ident": ident,
        })

    res = run_bass_kernel_spmd(
        nc, in_maps, core_ids=list(range(N_CORES)),
        trace=bool(int(os.environ.get("MHA_TRACE", "0"))),
    )
    LAST_RESULT = res

    ys = []
    for b in range(B):
        y = res.results[b * 4]["y_t"].astype(np.float64)
        for hg in range(1, 4):
            y += res.results[b * 4 + hg]["y_t"]
        ys.append(y.T)
    y = np.stack(ys) + np.asarray(wo_b, np.float64)[None, None, :]
    return y.astype(np.float32)
